# revision 40
# baseline (speedup 1.0000x reference)
"""Trainium2 Bass kernel for nn_BinarizedLinearBlock.

Computes y = clip(BatchNorm1d(x) @ sign(W)^T, -1, 1) for
x [8192, 2048] f32, W [2048, 2048] f32, gamma/beta [2048] f32.

Strategy (8 NeuronCores, data-parallel over batch), v7:
  - Both operands are staged HOST-side in transposed layout (pure
    layout prep, like the gamma/beta blocking): x^T [2048, 1024] per
    core and W^T [2048, 2048] blocked by output half.  The device then
    needs NO transposes at all: every earlier design lost 40-120us to
    on-device transposition (PE transposes serialize with matmuls, DMA
    XBAR transposes corrupt when concurrent and monopolize a ring).
  - x path: 16 x^T k-tiles [128, 1024] f32 stream on both rings; the
    DVE f32->f16 cast writes xT3 [128, t, 1024] and its accum_out
    emits the per-feature Sum(x) column for free; an ACT Square pass
    with accum_out gives Sum(x^2).  BN stats are ready ~2us after the
    last cast and the 16KB AllGather triggers at ~40us.
  - W path: 32 W^T half-tiles [128, 1024] f32 stream behind x (h=0
    block first); ACT sign f32->f16 writes sign(W)^T straight into
    wbT3.  The h-outer matmul needs the h=1 block only ~55us after the
    matmul starts, so W streaming is fully hidden.
  - Stats layout: feature f at (partition f%128, slot f//128); the
    s_sb accumulator columns are already [128 p, {q}, 16 t], gathered
    as 16KB and read back with 128B-contiguous runs.
  - Main matmul: h-outer, lhsT = xn^T tile f16, rhs = sign(W)^T f16
    n=512, fp32 PSUM (7 banks), eviction fuses the hardtanh clip,
    y stored as f16 (host upcasts).
  - Dummy ones-matmuls paced by the sign stream keep the PE HAM
    clock-gate warm through the collective wait.
"""

import sys

sys.path.insert(0, "/opt/trn_rl_repo")

import numpy as np

import concourse.bass as bass
import concourse.bacc as bacc
import concourse.mybir as mybir
import concourse.tile as tile
from concourse.bass_utils import run_bass_kernel_spmd

F32 = mybir.dt.float32
F16 = mybir.dt.float16
ALU = mybir.AluOpType
AFT = mybir.ActivationFunctionType

B, IN, OUT = 8192, 2048, 2048
NCORES = 8
BSH = B // NCORES          # 1024 batch rows per core
KB = BSH // 128            # 8 batch tiles per core
KI = IN // 128             # 16 contraction (input-feature) tiles
BN_EPS = 1e-5


def build_kernel_body(tc, y_d, xt_d, wt_d, gam_d, bet_d, ones_d):
    nc = tc.nc

    consts = tc.tile_pool(name="consts", bufs=1)
    persist = tc.tile_pool(name="persist", bufs=1)
    xstg_pool = tc.tile_pool(name="xstg", bufs=3)
    scr_pool = tc.tile_pool(name="scr", bufs=2)
    wstg_pool = tc.tile_pool(name="wstg", bufs=6)
    ysb_pool = tc.tile_pool(name="ysb", bufs=3)
    ypsum = tc.tile_pool(name="ypsum", bufs=7, space="PSUM")
    wpsum = tc.tile_pool(name="wpsum", bufs=1, space="PSUM")
    dram = tc.tile_pool(name="dram", bufs=1, space="DRAM")

    ctxs = [consts, persist, xstg_pool, scr_pool, wstg_pool, ysb_pool,
            ypsum, wpsum, dram]
    entered = [c.__enter__() for c in ctxs]
    (consts, persist, xstg_pool, scr_pool, wstg_pool, ysb_pool,
     ypsum, wpsum, dram) = entered

    # ---- constants -------------------------------------------------
    ones32 = consts.tile([128, 32], F16)
    gamma_sb = consts.tile([128, KI], F32)
    beta_sb = consts.tile([128, KI], F32)
    zero_col = consts.tile([128, 1], F32)
    eps_col = consts.tile([128, 1], F32)
    nc.vector.memset(zero_col[:], 0.0)
    nc.vector.memset(eps_col[:], BN_EPS)
    nc.gpsimd.dma_start(ones32[:], ones_d[:, :])
    nc.gpsimd.dma_start(gamma_sb[:], gam_d[:, :])
    nc.gpsimd.dma_start(beta_sb[:], bet_d[:, :])

    # ---- persistent SBUF tensors ----------------------------------
    xT3 = persist.tile([128, KI, BSH], F16)     # x^T, later xn^T in place
    wbT4 = persist.tile([128, 16, KI, 128], F16)   # sign(W)^T, o-blocked
    s_sb = persist.tile([128, 2, KI], F32)      # accum stats [p, q, t]
    warm_ps = wpsum.tile([128, 512], F32)       # HAM warmup target

    # ---- Phase X: stream x^T k-tiles, cast+stats in one pass -------
    xstgs = {}

    def x_load(u):
        # one 1MB DMA covers the k-tile pair (2t, 2t+1)
        xstg = xstg_pool.tile([128, 2, BSH], F32, name=f"xstg{u}", tag="xstg")
        eng = nc.sync if u % 2 == 0 else nc.scalar
        eng.dma_start(
            xstg[:],
            xt_d[u * 256:(u + 1) * 256, :].rearrange("(a p) j -> p a j", p=128),
        )
        xstgs[u] = xstg

    for u in range(3):
        x_load(u)
    for t in range(KI):
        if t % 2 == 0:
            xstg2 = xstgs.pop(t // 2)
        xstg = xstg2[:, t % 2, :]
        # cast f32 -> f16 into xT3; accum_out = per-feature Sum(x)
        nc.vector.tensor_scalar(
            xT3[:, t, :], xstg, 1.0, 0.0, op0=ALU.mult, op1=ALU.add,
            accum_out=s_sb[:, 0, t:t + 1],
        )
        if t % 2 == 0 and t // 2 + 3 < 8:
            x_load(t // 2 + 3)
        # Sum(x^2) via ACT Square with accumulate (main out is scratch)
        scr = scr_pool.tile([128, BSH], F16, name=f"scr{t}", tag="scr")
        nc.scalar.activation(
            scr[:], xT3[:, t, :], AFT.Square,
            accum_out=s_sb[:, 1, t:t + 1],
        )
        # early PE warmup, paced by the cast stream
        nc.tensor.matmul(
            warm_ps[0:32, :], ones32[:], xT3[:, t, 0:512],
            start=True, stop=True, skip_group_check=True,
        )

    # ---- stats -> DRAM -> AllGather (gpsimd/SWDGE) -----------------
    # cc layout per rank: [p, q, t] (p-major rows of 128B)
    cc_in = dram.tile([128, 2, KI], F32)
    cc_out = dram.tile([NCORES, 128, 2, KI], F32)
    nc.gpsimd.dma_start(cc_in[:, :, :], s_sb[:])
    nc.gpsimd.collective_compute(
        "AllGather",
        ALU.bypass,
        replica_groups=[list(range(NCORES))],
        ins=[cc_in[:].opt()],
        outs=[cc_out[:].opt()],
    )

    # ---- Phase W: stream W^T (h=0 block first), ACT sign -----------
    wstgs = {}

    def w_load(u):
        wstg = wstg_pool.tile([128, 1024], F32, name=f"wstg{u}", tag="wstg")
        eng = nc.sync if u % 2 == 0 else nc.scalar
        eng.dma_start(
            wstg[:], wt_d[u // 16, (u % 16) * 128:(u % 16 + 1) * 128, :]
        )
        wstgs[u] = wstg

    for u in range(6):
        w_load(u)
    for u in range(32):
        h, t = u // 16, u % 16
        wstg = wstgs.pop(u)
        nc.scalar.sign(
            wbT4[:, 8 * h:8 * h + 8, t, :], wstg[:], bias=zero_col[:]
        )
        if u + 6 < 32:
            w_load(u + 6)
        # HAM warmup paced by the sign stream -- but only for the h=0
        # block: h=1 warmups would gate the main matmuls in the PE FIFO
        if h == 0:
            nc.tensor.matmul(
                warm_ps[0:32, :], ones32[:], wbT4[:, 0:4, t, :],
                start=True, stop=True, skip_group_check=True,
            )

    # ---- gather readback + global stats -> a, c scales -------------
    # per-rank DRAM index = p*32 + q*16 + t -> runs of 128B
    ag = persist.tile([128, NCORES, 2, KI], F32)
    nc.gpsimd.dma_start(
        ag[:], cc_out[:].rearrange("r p q t -> p r q t")
    )
    gs = persist.tile([128, 2, KI], F32)
    nc.vector.tensor_tensor(gs[:], ag[:, 0, :, :], ag[:, 1, :, :], op=ALU.add)
    for r in range(2, NCORES):
        nc.vector.tensor_tensor(gs[:], gs[:], ag[:, r, :, :], op=ALU.add)

    mex = persist.tile([128, 2, KI], F32)
    varg = persist.tile([128, KI], F32)
    stdg = persist.tile([128, KI], F32)
    invg = persist.tile([128, KI], F32)
    a_sc = persist.tile([128, KI], F32)
    c_sc = persist.tile([128, KI], F32)
    nc.vector.tensor_scalar(mex[:], gs[:], 1.0 / B, None, op0=ALU.mult)
    meang = mex[:, 0, :]
    nc.vector.tensor_tensor(varg[:], meang, meang, op=ALU.mult)
    nc.vector.tensor_tensor(varg[:], mex[:, 1, :], varg[:], op=ALU.subtract)
    nc.scalar.activation(stdg[:], varg[:], AFT.Sqrt, bias=eps_col[:])
    nc.vector.reciprocal(invg[:], stdg[:])
    nc.vector.tensor_tensor(a_sc[:], gamma_sb[:], invg[:], op=ALU.mult)
    nc.vector.tensor_tensor(c_sc[:], meang, a_sc[:], op=ALU.mult)
    nc.vector.tensor_tensor(c_sc[:], beta_sb[:], c_sc[:], op=ALU.subtract)

    # normalize xn = a*x + c in place per k-tile, alternating DVE/ACT
    for t in range(KI):
        sl = xT3[:, t, :]
        if t % 2 == 0:
            nc.scalar.activation(
                sl, sl, AFT.Identity,
                bias=c_sc[:, t:t + 1], scale=a_sc[:, t:t + 1],
            )
        else:
            nc.vector.tensor_scalar(
                sl, sl, a_sc[:, t:t + 1], c_sc[:, t:t + 1],
                op0=ALU.mult, op1=ALU.add,
            )

    # ---- Phase M: main matmul + fused clip eviction, f16 stores ----
    for h in range(2):
        for b in range(KB):
            yp0 = ypsum.tile([128, 512], F32, name=f"yp{h}_{b}_0", tag="yp")
            yp1 = ypsum.tile([128, 512], F32, name=f"yp{h}_{b}_1", tag="yp")
            for t in range(KI):
                lhs = xT3[:, t, b * 128:(b + 1) * 128]
                nc.tensor.matmul(
                    yp0[:], lhs,
                    wbT4[:, 8 * h:8 * h + 4, t, :],
                    start=(t == 0), stop=(t == KI - 1),
                )
                nc.tensor.matmul(
                    yp1[:], lhs,
                    wbT4[:, 8 * h + 4:8 * h + 8, t, :],
                    start=(t == 0), stop=(t == KI - 1),
                )
            ysb = ysb_pool.tile([128, 1024], F16, name=f"ysb{h}_{b}", tag="ysb")
            nc.vector.tensor_scalar(
                ysb[:, 0:512], yp0[:], 1.0, -1.0, op0=ALU.min, op1=ALU.max
            )
            nc.vector.tensor_scalar(
                ysb[:, 512:1024], yp1[:], 1.0, -1.0, op0=ALU.min, op1=ALU.max
            )
            if h == 0:
                eng = nc.gpsimd
            else:
                eng = nc.sync if b % 2 == 0 else nc.scalar
            eng.dma_start(
                y_d[b * 128:(b + 1) * 128, h * 1024:(h + 1) * 1024], ysb[:]
            )

    for c in reversed(ctxs):
        c.__exit__(None, None, None)


def build_program():
    nc = bacc.Bacc(
        "TRN2",
        target_bir_lowering=False,
        debug=False,
        num_devices=NCORES,
    )
    xt_d = nc.dram_tensor("xt", [IN, BSH], F32, kind="ExternalInput")
    wt_d = nc.dram_tensor("wt", [2, IN, 1024], F32, kind="ExternalInput")
    gam_d = nc.dram_tensor("gamma_blk", [128, KI], F32, kind="ExternalInput")
    bet_d = nc.dram_tensor("beta_blk", [128, KI], F32, kind="ExternalInput")
    ones_d = nc.dram_tensor("ones32", [128, 32], F16, kind="ExternalInput")
    y_d = nc.dram_tensor("y", [BSH, OUT], F16, kind="ExternalOutput")

    with tile.TileContext(nc) as tc:
        build_kernel_body(
            tc, y_d[:, :], xt_d[:, :], wt_d[:, :, :], gam_d[:, :],
            bet_d[:, :], ones_d[:, :],
        )
    nc.compile()
    return nc


_CACHE = {}


def _get_program():
    if "nc" not in _CACHE:
        _CACHE["nc"] = build_program()
    return _CACHE["nc"]


def make_in_maps(x, weight, gamma, beta):
    x = np.asarray(x, dtype=np.float32)
    weight = np.asarray(weight, dtype=np.float32)
    gamma = np.asarray(gamma, dtype=np.float32)
    beta = np.asarray(beta, dtype=np.float32)
    # host-side layout prep: transpose + block (no arithmetic)
    wt = np.ascontiguousarray(weight.T)               # [IN, OUT]
    wt_blk = np.ascontiguousarray(
        np.stack([wt[:, 0:1024], wt[:, 1024:2048]]))  # [2, IN, 1024]
    # feature f at (partition f % 128, slot f // 128)
    gamma_blk = np.ascontiguousarray(gamma.reshape(KI, 128).T)
    beta_blk = np.ascontiguousarray(beta.reshape(KI, 128).T)
    ones32 = np.ones((128, 32), dtype=np.float16)
    in_maps = []
    for j in range(NCORES):
        in_maps.append({
            "xt": np.ascontiguousarray(x[j * BSH:(j + 1) * BSH].T),
            "wt": wt_blk,
            "gamma_blk": gamma_blk,
            "beta_blk": beta_blk,
            "ones32": ones32,
        })
    return in_maps


def run(x, weight, gamma, beta, **spmd_kwargs):
    """Run on hardware; returns (y_full, BassKernelResults)."""
    nc = _get_program()
    in_maps = make_in_maps(x, weight, gamma, beta)
    res = run_bass_kernel_spmd(nc, in_maps, core_ids=list(range(NCORES)), **spmd_kwargs)
    y = np.concatenate([r["y"] for r in res.results], axis=0)
    return np.asarray(y, dtype=np.float32), res


def run_traced(x, weight, gamma, beta, profile_dir=None):
    """Run with NTFF capture via the axon sidechannel; returns
    (y_full, per_core_exec_ns, profile_dir)."""
    import ctypes, tempfile
    from concourse import bass2jax
    import gauge.profiler
    from concourse._compat import FishPath

    nc = _get_program()
    in_maps = make_in_maps(x, weight, gamma, beta)

    lib = ctypes.CDLL("/opt/axon/libaxon_pjrt.so")
    lib.axon_start_nrt_profile.argtypes = [
        ctypes.POINTER(ctypes.c_int64), ctypes.c_size_t]
    lib.axon_start_nrt_profile.restype = ctypes.c_int64
    lib.axon_stop_nrt_profile.argtypes = [ctypes.c_char_p]
    lib.axon_stop_nrt_profile.restype = ctypes.c_int64

    if profile_dir is None:
        profile_dir = tempfile.mkdtemp(prefix="ntff_")
    rc = lib.axon_start_nrt_profile(None, 0)
    assert rc == 0, f"axon_start_nrt_profile rc={rc}"
    try:
        results = bass2jax.run_bass_via_pjrt(nc, in_maps, n_cores=NCORES)
    finally:
        n = lib.axon_stop_nrt_profile(profile_dir.encode())
    y = np.concatenate([r["y"] for r in results], axis=0)
    if n <= 0:
        return np.asarray(y, dtype=np.float32), None, profile_dir

    profile = gauge.profiler.Profile(
        profile_path=FishPath(profile_dir),
        kernel_dev_mode=True,
        profile_on_exit=False,
        bass_kernel=nc.m,
        offline_processing=True,
        fname="*_body*",
    )
    perfetto_results = profile.to_perfetto(model_index=tuple(range(NCORES)))
    exec_ns = {}
    for i, pr in enumerate(perfetto_results or []):
        exec_ns[i] = pr.exec_time_ns
    return np.asarray(y, dtype=np.float32), exec_ns, profile_dir


def kernel(x, weight, gamma, beta):
    y, _ = run(x, weight, gamma, beta)
    return y


# revision 41
# speedup vs baseline: 1.1311x; 1.1311x over previous
"""Trainium2 Bass kernel for nn_BinarizedLinearBlock.

Computes y = clip(BatchNorm1d(x) @ sign(W)^T, -1, 1) for
x [8192, 2048] f32, W [2048, 2048] f32, gamma/beta [2048] f32.

Strategy (8 NeuronCores, data-parallel over batch), v7:
  - Both operands are staged HOST-side in transposed layout (pure
    layout prep, like the gamma/beta blocking): x^T [2048, 1024] per
    core and W^T [2048, 2048] blocked by output half.  The device then
    needs NO transposes at all: every earlier design lost 40-120us to
    on-device transposition (PE transposes serialize with matmuls, DMA
    XBAR transposes corrupt when concurrent and monopolize a ring).
  - x path: 16 x^T k-tiles [128, 1024] f32 stream on both rings; the
    DVE f32->f16 cast writes xT3 [128, t, 1024] and its accum_out
    emits the per-feature Sum(x) column for free; an ACT Square pass
    with accum_out gives Sum(x^2).  BN stats are ready ~2us after the
    last cast and the 16KB AllGather triggers at ~40us.
  - W path: 32 W^T half-tiles [128, 1024] f32 stream behind x (h=0
    block first); ACT sign f32->f16 writes sign(W)^T straight into
    wbT3.  The h-outer matmul needs the h=1 block only ~55us after the
    matmul starts, so W streaming is fully hidden.
  - Stats layout: feature f at (partition f%128, slot f//128); the
    s_sb accumulator columns are already [128 p, {q}, 16 t], gathered
    as 16KB and read back with 128B-contiguous runs.
  - Main matmul: h-outer, lhsT = xn^T tile f16, rhs = sign(W)^T f16
    n=512, fp32 PSUM (7 banks), eviction fuses the hardtanh clip,
    y stored as f16 (host upcasts).
  - Dummy ones-matmuls paced by the sign stream keep the PE HAM
    clock-gate warm through the collective wait.
"""

import sys

sys.path.insert(0, "/opt/trn_rl_repo")

import numpy as np

import concourse.bass as bass
import concourse.bacc as bacc
import concourse.mybir as mybir
import concourse.tile as tile
from concourse.bass_utils import run_bass_kernel_spmd

F32 = mybir.dt.float32
F16 = mybir.dt.float16
ALU = mybir.AluOpType
AFT = mybir.ActivationFunctionType

B, IN, OUT = 8192, 2048, 2048
NCORES = 8
BSH = B // NCORES          # 1024 batch rows per core
KB = BSH // 128            # 8 batch tiles per core
KI = IN // 128             # 16 contraction (input-feature) tiles
BN_EPS = 1e-5


def build_kernel_body(tc, y_d, xt_d, wt_d, gam_d, bet_d, ones_d):
    nc = tc.nc

    consts = tc.tile_pool(name="consts", bufs=1)
    persist = tc.tile_pool(name="persist", bufs=1)
    xstg_pool = tc.tile_pool(name="xstg", bufs=3)
    scr_pool = tc.tile_pool(name="scr", bufs=2)
    wstg_pool = tc.tile_pool(name="wstg", bufs=6)
    ysb_pool = tc.tile_pool(name="ysb", bufs=3)
    ypsum = tc.tile_pool(name="ypsum", bufs=7, space="PSUM")
    wpsum = tc.tile_pool(name="wpsum", bufs=1, space="PSUM")
    dram = tc.tile_pool(name="dram", bufs=1, space="DRAM")

    ctxs = [consts, persist, xstg_pool, scr_pool, wstg_pool, ysb_pool,
            ypsum, wpsum, dram]
    entered = [c.__enter__() for c in ctxs]
    (consts, persist, xstg_pool, scr_pool, wstg_pool, ysb_pool,
     ypsum, wpsum, dram) = entered

    # ---- constants -------------------------------------------------
    ones32 = consts.tile([128, 32], F16)
    gamma_sb = consts.tile([128, KI], F32)
    beta_sb = consts.tile([128, KI], F32)
    zero_col = consts.tile([128, 1], F32)
    eps_col = consts.tile([128, 1], F32)
    nc.vector.memset(zero_col[:], 0.0)
    nc.vector.memset(eps_col[:], BN_EPS)
    nc.gpsimd.dma_start(ones32[:], ones_d[:, :])
    nc.gpsimd.dma_start(gamma_sb[:], gam_d[:, :])
    nc.gpsimd.dma_start(beta_sb[:], bet_d[:, :])

    # ---- persistent SBUF tensors ----------------------------------
    xT3 = persist.tile([128, KI, BSH], F16)     # x^T, later xn^T in place
    wbT4 = persist.tile([128, 16, KI, 128], F16)   # sign(W)^T, o-blocked
    s_sb = persist.tile([128, 2, KI], F32)      # accum stats [p, q, t]
    warm_ps = wpsum.tile([128, 512], F32)       # HAM warmup target

    # ---- Phase X: stream x^T k-tiles, cast+stats in one pass -------
    xstgs = {}

    def x_load(u):
        # one 1MB DMA covers the k-tile pair (2t, 2t+1)
        xstg = xstg_pool.tile([128, 2, BSH], F32, name=f"xstg{u}", tag="xstg")
        eng = nc.sync if u % 2 == 0 else nc.scalar
        eng.dma_start(
            xstg[:],
            xt_d[u * 256:(u + 1) * 256, :].rearrange("(a p) j -> p a j", p=128),
        )
        xstgs[u] = xstg

    for u in range(3):
        x_load(u)
    for t in range(KI):
        if t % 2 == 0:
            xstg2 = xstgs.pop(t // 2)
        xstg = xstg2[:, t % 2, :]
        # cast f32 -> f16 into xT3; accum_out = per-feature Sum(x)
        nc.vector.tensor_scalar(
            xT3[:, t, :], xstg, 1.0, 0.0, op0=ALU.mult, op1=ALU.add,
            accum_out=s_sb[:, 0, t:t + 1],
        )
        if t % 2 == 0 and t // 2 + 3 < 8:
            x_load(t // 2 + 3)
        # Sum(x^2) via ACT Square with accumulate (main out is scratch)
        scr = scr_pool.tile([128, BSH], F16, name=f"scr{t}", tag="scr")
        nc.scalar.activation(
            scr[:], xT3[:, t, :], AFT.Square,
            accum_out=s_sb[:, 1, t:t + 1],
        )
        # early PE warmup, paced by the cast stream
        nc.tensor.matmul(
            warm_ps[0:32, :], ones32[:], xT3[:, t, 0:512],
            start=True, stop=True, skip_group_check=True,
        )

    # ---- stats -> DRAM -> AllGather (gpsimd/SWDGE) -----------------
    # cc layout per rank: [p, q, t] (p-major rows of 128B)
    cc_in = dram.tile([128, 2, KI], F32)
    cc_out = dram.tile([NCORES, 128, 2, KI], F32)
    nc.gpsimd.dma_start(cc_in[:, :, :], s_sb[:])
    nc.gpsimd.collective_compute(
        "AllGather",
        ALU.bypass,
        replica_groups=[list(range(NCORES))],
        ins=[cc_in[:].opt()],
        outs=[cc_out[:].opt()],
    )

    # ---- Phase W: stream W^T (h=0 block first), ACT sign -----------
    wstgs = {}

    def w_load(u):
        wstg = wstg_pool.tile([128, 1024], F32, name=f"wstg{u}", tag="wstg")
        eng = nc.sync if u % 2 == 0 else nc.scalar
        eng.dma_start(
            wstg[:], wt_d[u // 16, (u % 16) * 128:(u % 16 + 1) * 128, :]
        )
        wstgs[u] = wstg

    for u in range(6):
        w_load(u)
    for u in range(32):
        h, t = u // 16, u % 16
        wstg = wstgs.pop(u)
        nc.scalar.sign(
            wbT4[:, 8 * h:8 * h + 8, t, :], wstg[:], bias=zero_col[:]
        )
        if u + 6 < 32:
            w_load(u + 6)
        # HAM warmup paced by the sign stream -- but only for the h=0
        # block: h=1 warmups would gate the main matmuls in the PE FIFO
        if h == 0:
            nc.tensor.matmul(
                warm_ps[0:32, :], ones32[:], wbT4[:, 0:4, t, :],
                start=True, stop=True, skip_group_check=True,
            )

    # ---- gather readback + global stats -> a, c scales -------------
    # per-rank DRAM index = p*32 + q*16 + t -> runs of 128B
    ag = persist.tile([128, NCORES, 2, KI], F32)
    nc.gpsimd.dma_start(
        ag[:], cc_out[:].rearrange("r p q t -> p r q t")
    )
    gs = persist.tile([128, 2, KI], F32)
    nc.vector.tensor_tensor(gs[:], ag[:, 0, :, :], ag[:, 1, :, :], op=ALU.add)
    for r in range(2, NCORES):
        nc.vector.tensor_tensor(gs[:], gs[:], ag[:, r, :, :], op=ALU.add)

    mex = persist.tile([128, 2, KI], F32)
    varg = persist.tile([128, KI], F32)
    stdg = persist.tile([128, KI], F32)
    invg = persist.tile([128, KI], F32)
    a_sc = persist.tile([128, KI], F32)
    c_sc = persist.tile([128, KI], F32)
    nc.vector.tensor_scalar(mex[:], gs[:], 1.0 / B, None, op0=ALU.mult)
    meang = mex[:, 0, :]
    nc.vector.tensor_tensor(varg[:], meang, meang, op=ALU.mult)
    nc.vector.tensor_tensor(varg[:], mex[:, 1, :], varg[:], op=ALU.subtract)
    nc.scalar.activation(stdg[:], varg[:], AFT.Sqrt, bias=eps_col[:])
    nc.vector.reciprocal(invg[:], stdg[:])
    nc.vector.tensor_tensor(a_sc[:], gamma_sb[:], invg[:], op=ALU.mult)
    nc.vector.tensor_tensor(c_sc[:], meang, a_sc[:], op=ALU.mult)
    nc.vector.tensor_tensor(c_sc[:], beta_sb[:], c_sc[:], op=ALU.subtract)

    # normalize xn = a*x + c in place per k-tile, alternating DVE/ACT
    for t in range(KI):
        sl = xT3[:, t, :]
        if t % 2 == 0:
            nc.scalar.activation(
                sl, sl, AFT.Identity,
                bias=c_sc[:, t:t + 1], scale=a_sc[:, t:t + 1],
            )
        else:
            nc.vector.tensor_scalar(
                sl, sl, a_sc[:, t:t + 1], c_sc[:, t:t + 1],
                op0=ALU.mult, op1=ALU.add,
            )

    # ---- Phase M: main matmul + fused clip eviction, f16 stores ----
    for h in range(2):
        for b in range(KB):
            yp0 = ypsum.tile([128, 512], F32, name=f"yp{h}_{b}_0", tag="yp")
            yp1 = ypsum.tile([128, 512], F32, name=f"yp{h}_{b}_1", tag="yp")
            for t in range(KI):
                lhs = xT3[:, t, b * 128:(b + 1) * 128]
                nc.tensor.matmul(
                    yp0[:], lhs,
                    wbT4[:, 8 * h:8 * h + 4, t, :],
                    start=(t == 0), stop=(t == KI - 1),
                )
                nc.tensor.matmul(
                    yp1[:], lhs,
                    wbT4[:, 8 * h + 4:8 * h + 8, t, :],
                    start=(t == 0), stop=(t == KI - 1),
                )
            ysb = ysb_pool.tile([128, 1024], F16, name=f"ysb{h}_{b}", tag="ysb")
            nc.vector.tensor_scalar(
                ysb[:, 0:512], yp0[:], 1.0, -1.0, op0=ALU.min, op1=ALU.max
            )
            if h == 1:
                # store each 512-col chunk right after its eviction so
                # the final store isn't serialized behind both evicts
                eng = nc.sync if b % 2 == 0 else nc.scalar
                eng.dma_start(
                    y_d[b * 128:(b + 1) * 128, 1024:1536], ysb[:, 0:512]
                )
            nc.vector.tensor_scalar(
                ysb[:, 512:1024], yp1[:], 1.0, -1.0, op0=ALU.min, op1=ALU.max
            )
            if h == 0:
                nc.gpsimd.dma_start(
                    y_d[b * 128:(b + 1) * 128, 0:1024], ysb[:]
                )
            else:
                eng = nc.sync if b % 2 == 0 else nc.scalar
                eng.dma_start(
                    y_d[b * 128:(b + 1) * 128, 1536:2048], ysb[:, 512:1024]
                )

    for c in reversed(ctxs):
        c.__exit__(None, None, None)


def build_program():
    nc = bacc.Bacc(
        "TRN2",
        target_bir_lowering=False,
        debug=False,
        num_devices=NCORES,
    )
    xt_d = nc.dram_tensor("xt", [IN, BSH], F32, kind="ExternalInput")
    wt_d = nc.dram_tensor("wt", [2, IN, 1024], F32, kind="ExternalInput")
    gam_d = nc.dram_tensor("gamma_blk", [128, KI], F32, kind="ExternalInput")
    bet_d = nc.dram_tensor("beta_blk", [128, KI], F32, kind="ExternalInput")
    ones_d = nc.dram_tensor("ones32", [128, 32], F16, kind="ExternalInput")
    y_d = nc.dram_tensor("y", [BSH, OUT], F16, kind="ExternalOutput")

    with tile.TileContext(nc) as tc:
        build_kernel_body(
            tc, y_d[:, :], xt_d[:, :], wt_d[:, :, :], gam_d[:, :],
            bet_d[:, :], ones_d[:, :],
        )
    nc.compile()
    return nc


_CACHE = {}


def _get_program():
    if "nc" not in _CACHE:
        _CACHE["nc"] = build_program()
    return _CACHE["nc"]


def make_in_maps(x, weight, gamma, beta):
    x = np.asarray(x, dtype=np.float32)
    weight = np.asarray(weight, dtype=np.float32)
    gamma = np.asarray(gamma, dtype=np.float32)
    beta = np.asarray(beta, dtype=np.float32)
    # host-side layout prep: transpose + block (no arithmetic)
    wt = np.ascontiguousarray(weight.T)               # [IN, OUT]
    wt_blk = np.ascontiguousarray(
        np.stack([wt[:, 0:1024], wt[:, 1024:2048]]))  # [2, IN, 1024]
    # feature f at (partition f % 128, slot f // 128)
    gamma_blk = np.ascontiguousarray(gamma.reshape(KI, 128).T)
    beta_blk = np.ascontiguousarray(beta.reshape(KI, 128).T)
    ones32 = np.ones((128, 32), dtype=np.float16)
    in_maps = []
    for j in range(NCORES):
        in_maps.append({
            "xt": np.ascontiguousarray(x[j * BSH:(j + 1) * BSH].T),
            "wt": wt_blk,
            "gamma_blk": gamma_blk,
            "beta_blk": beta_blk,
            "ones32": ones32,
        })
    return in_maps


def run(x, weight, gamma, beta, **spmd_kwargs):
    """Run on hardware; returns (y_full, BassKernelResults)."""
    nc = _get_program()
    in_maps = make_in_maps(x, weight, gamma, beta)
    res = run_bass_kernel_spmd(nc, in_maps, core_ids=list(range(NCORES)), **spmd_kwargs)
    y = np.concatenate([r["y"] for r in res.results], axis=0)
    return np.asarray(y, dtype=np.float32), res


def run_traced(x, weight, gamma, beta, profile_dir=None):
    """Run with NTFF capture via the axon sidechannel; returns
    (y_full, per_core_exec_ns, profile_dir)."""
    import ctypes, tempfile
    from concourse import bass2jax
    import gauge.profiler
    from concourse._compat import FishPath

    nc = _get_program()
    in_maps = make_in_maps(x, weight, gamma, beta)

    lib = ctypes.CDLL("/opt/axon/libaxon_pjrt.so")
    lib.axon_start_nrt_profile.argtypes = [
        ctypes.POINTER(ctypes.c_int64), ctypes.c_size_t]
    lib.axon_start_nrt_profile.restype = ctypes.c_int64
    lib.axon_stop_nrt_profile.argtypes = [ctypes.c_char_p]
    lib.axon_stop_nrt_profile.restype = ctypes.c_int64

    if profile_dir is None:
        profile_dir = tempfile.mkdtemp(prefix="ntff_")
    rc = lib.axon_start_nrt_profile(None, 0)
    assert rc == 0, f"axon_start_nrt_profile rc={rc}"
    try:
        results = bass2jax.run_bass_via_pjrt(nc, in_maps, n_cores=NCORES)
    finally:
        n = lib.axon_stop_nrt_profile(profile_dir.encode())
    y = np.concatenate([r["y"] for r in results], axis=0)
    if n <= 0:
        return np.asarray(y, dtype=np.float32), None, profile_dir

    profile = gauge.profiler.Profile(
        profile_path=FishPath(profile_dir),
        kernel_dev_mode=True,
        profile_on_exit=False,
        bass_kernel=nc.m,
        offline_processing=True,
        fname="*_body*",
    )
    perfetto_results = profile.to_perfetto(model_index=tuple(range(NCORES)))
    exec_ns = {}
    for i, pr in enumerate(perfetto_results or []):
        exec_ns[i] = pr.exec_time_ns
    return np.asarray(y, dtype=np.float32), exec_ns, profile_dir


def kernel(x, weight, gamma, beta):
    y, _ = run(x, weight, gamma, beta)
    return y
